# revision 2
# baseline (speedup 1.0000x reference)
"""Energy-distance kernel for 8 TRN2 NeuronCores (Bass/Tile).

Problem: x1 [16, 256, 128], x2 [16, 256, 128] fp32 ->
    energy[b] = mean_{i,j} ||x1[b,i]-x2[b,j]||_1
                - 0.5 * (mean ||x2-x2||_1 + mean ||x1-x1||_1)

Sharding: pure data parallel over the batch dim (2 batches per core).

Algorithm (per core): the O(N^2 D) pairwise L1 sums decompose per
feature dimension d into 1-D order statistics. For each (batch, d), the
merged array w = concat(u, v) (512 values, u = x1[b,:,d], v = x2[b,:,d])
is sorted with a 45-stage bitonic network on the vector engine (128 d's
in parallel across partitions). Origin (u vs v) rides in the key
mantissa LSB. With s = +-1 origin signs and G = inclusive cumsum(s):
    E(b,d) = sum(w) - 2 * sum(w * s * G)
    energy[b] = sum_d E(b,d) / (N*M)
The device returns R1 = sum(w*s*G) and R2 = sum(w) per (d, b); the host
combines them in float64.

FP16_KEYS=False sorts fp32 keys (rel err ~5e-5, matching the fp32
reference's own rounding); True sorts fp16 keys with the NB batch
arrays interleaved for DVE 2x mode (~30% faster, rel err ~6e-4).
"""
import numpy as np
from contextlib import ExitStack

from concourse import bacc, mybir, masks
from concourse.tile import TileContext
from concourse.bass_utils import run_bass_kernel_spmd

F32 = mybir.dt.float32
F16 = mybir.dt.float16
I32 = mybir.dt.int32
I16 = mybir.dt.int16
Alu = mybir.AluOpType

NCORES = 8
B = 16        # total batches
NB = 2        # batches per core
NPTS = 256    # points per set (N == M)
L = 2 * NPTS  # merged array length per (b, d)
WFREE = NB * L
D = 128

FP16_KEYS = False


def _bitonic_stages(length):
    k = 2
    while k <= length:
        yield ("rev", k)
        j = k // 4
        while j >= 1:
            yield ("reg", j)
            j //= 2
        k *= 2


def _build_nc(fp16=FP16_KEYS):
    interleave = fp16
    KD = F16 if fp16 else F32
    KI = I16 if fp16 else I32
    nc = bacc.Bacc("TRN2", target_bir_lowering=False, debug=False,
                   num_devices=NCORES)
    x1 = nc.dram_tensor("x1", [NB, NPTS, D], F32, kind="ExternalInput").ap()
    x2 = nc.dram_tensor("x2", [NB, NPTS, D], F32, kind="ExternalInput").ap()
    r1 = nc.dram_tensor("r1", [D, NB], F32, kind="ExternalOutput").ap()
    r2 = nc.dram_tensor("r2", [D, NB], F32, kind="ExternalOutput").ap()

    with TileContext(nc) as tc, ExitStack() as ctx:
        cpool = ctx.enter_context(tc.tile_pool(name="consts", bufs=1))
        wpool = ctx.enter_context(tc.tile_pool(name="work", bufs=1))
        xpool = ctx.enter_context(tc.tile_pool(name="xload", bufs=8))
        ppool = ctx.enter_context(tc.tile_pool(name="ps", bufs=2, space="PSUM"))

        ident = cpool.tile([128, 128], F32, tag="ident")
        masks.make_identity(nc, ident[:])

        wa = wpool.tile([128, WFREE], KD, tag="wa")
        wb = wpool.tile([128, WFREE], KD, tag="wb")

        def arr_slice(buf, b):
            if interleave:
                return buf[:].rearrange("p (l b) -> p l b", b=NB)[:, :, b]
            return buf[:, b * L:(b + 1) * L]

        # ---- load, transpose to [d, n], pack into wa ----
        for b in range(NB):
            ps = ppool.tile([128, L], F32, tag="psb")
            for half, x in ((0, x1), (1, x2)):
                for t in range(NPTS // 128):
                    xt = xpool.tile([128, 128], F32, tag="xt")
                    nc.sync.dma_start(out=xt[:],
                                      in_=x[b, t * 128:(t + 1) * 128, :])
                    c0 = (half * (NPTS // 128) + t) * 128
                    nc.tensor.transpose(ps[:, c0:c0 + 128], xt[:], ident[:])
            nc.scalar.copy(out=arr_slice(wa, b), in_=ps[:])

        # ---- LSB origin tag: u -> LSB 0, v -> LSB 1 ----
        wi = wa[:].bitcast(KI)
        if interleave:
            u_ap, v_ap = wi[:, :NB * NPTS], wi[:, NB * NPTS:]
        else:
            r = wi.rearrange("p (b h n) -> p b h n", b=NB, h=2)
            u_ap, v_ap = r[:, :, 0, :], r[:, :, 1, :]
        nc.vector.tensor_scalar(out=u_ap, in0=u_ap, scalar1=-2,
                                scalar2=None, op0=Alu.bitwise_and)
        nc.vector.tensor_scalar(out=v_ap, in0=v_ap, scalar1=1,
                                scalar2=None, op0=Alu.bitwise_or)

        # ---- bitonic sort (ping-pong wa <-> wb) ----
        IL = NB if interleave else 1
        bufs = [wa, wb]
        cur = 0
        for kind, p in _bitonic_stages(L):
            src = bufs[cur][:]
            dst = bufs[1 - cur][:]
            if kind == "reg":
                m = IL * p
                rs = src.rearrange("p (n t m) -> p n t m", t=2, m=m)
                rd = dst.rearrange("p (n t m) -> p n t m", t=2, m=m)
                lo_s, hi_s = rs[:, :, 0, :], rs[:, :, 1, :]
                lo_d, hi_d = rd[:, :, 0, :], rd[:, :, 1, :]
            elif interleave:
                rs = src.rearrange("p (n t x b) -> p n t x b",
                                   t=2, x=p // 2, b=IL)
                rd = dst.rearrange("p (n t x b) -> p n t x b",
                                   t=2, x=p // 2, b=IL)
                lo_s, hi_s = rs[:, :, 0, :, :], rs[:, :, 1, ::-1, :]
                lo_d, hi_d = rd[:, :, 0, :, :], rd[:, :, 1, ::-1, :]
            else:
                rs = src.rearrange("p (n t h) -> p n t h", t=2, h=p // 2)
                rd = dst.rearrange("p (n t h) -> p n t h", t=2, h=p // 2)
                lo_s, hi_s = rs[:, :, 0, :], rs[:, :, 1, ::-1]
                lo_d, hi_d = rd[:, :, 0, :], rd[:, :, 1, ::-1]
            nc.vector.tensor_tensor(out=lo_d, in0=lo_s, in1=hi_s, op=Alu.min)
            nc.vector.tensor_tensor(out=hi_d, in0=lo_s, in1=hi_s, op=Alu.max)
            cur = 1 - cur

        ws = bufs[cur]       # sorted keys (LSB-tagged)
        aux = bufs[1 - cur]  # reuse the other buffer for the mask

        # ---- origin signs s = 1 - 2*(w & 1) ----
        mi = aux[:].bitcast(KI)
        nc.vector.tensor_scalar(out=mi, in0=ws[:].bitcast(KI), scalar1=1,
                                scalar2=None, op0=Alu.bitwise_and)
        mf = wpool.tile([128, WFREE], KD, tag="mf")
        nc.vector.tensor_copy(out=mf[:], in_=mi)
        s_t = wpool.tile([128, WFREE], KD, tag="s_t")
        nc.vector.tensor_scalar(out=s_t[:], in0=mf[:], scalar1=-2.0,
                                scalar2=1.0, op0=Alu.mult, op1=Alu.add)

        # ---- G = inclusive cumsum(s) per (b) array ----
        g_t = wpool.tile([128, WFREE], KD, tag="g_t")
        for b in range(NB):
            nc.vector.tensor_tensor_scan(
                out=arr_slice(g_t, b), data0=arr_slice(s_t, b),
                data1=arr_slice(s_t, b),
                initial=0.0, op0=Alu.add, op1=Alu.bypass)

        # ---- R1 = sum(w*s*G), R2 = sum(w) per (d, b); reduce on ACT ----
        t1 = wpool.tile([128, WFREE], KD, tag="t1")
        nc.vector.tensor_mul(out=t1[:], in0=ws[:], in1=s_t[:])
        t2 = wpool.tile([128, WFREE], KD, tag="t2")
        nc.vector.tensor_mul(out=t2[:], in0=t1[:], in1=g_t[:])
        r1sb = wpool.tile([128, NB], F32, tag="r1sb")
        r2sb = wpool.tile([128, NB], F32, tag="r2sb")
        trash = wpool.tile([128, L], KD, tag="trash")
        Act = mybir.ActivationFunctionType
        for b in range(NB):
            nc.scalar.activation(out=trash[:], in_=arr_slice(ws, b),
                                 func=Act.Copy, accum_out=r2sb[:, b:b + 1])
        for b in range(NB):
            nc.scalar.activation(out=trash[:], in_=arr_slice(t2, b),
                                 func=Act.Copy, accum_out=r1sb[:, b:b + 1])

        nc.sync.dma_start(out=r1, in_=r1sb[:])
        nc.sync.dma_start(out=r2, in_=r2sb[:])

    nc.finalize()
    return nc


_NC_CACHE = {}


def _get_nc():
    if "nc" not in _NC_CACHE:
        _NC_CACHE["nc"] = _build_nc()
    return _NC_CACHE["nc"]


def kernel(x1, x2):
    """x1, x2: [16, 256, 128] fp32 -> energy distances [16] fp32."""
    x1 = np.ascontiguousarray(np.asarray(x1, dtype=np.float32))
    x2 = np.ascontiguousarray(np.asarray(x2, dtype=np.float32))
    nc = _get_nc()
    core_ids = list(range(NCORES))
    in_maps = []
    for c in core_ids:
        sl = slice(c * NB, (c + 1) * NB)
        in_maps.append({"x1": np.ascontiguousarray(x1[sl]),
                        "x2": np.ascontiguousarray(x2[sl])})
    res = run_bass_kernel_spmd(nc, in_maps, core_ids)
    energy = np.zeros(B, dtype=np.float64)
    for c in core_ids:
        r1 = res.results[c]["r1"].astype(np.float64)  # [D, NB]
        r2 = res.results[c]["r2"].astype(np.float64)
        e_bd = r2 - 2.0 * r1
        energy[c * NB:(c + 1) * NB] = e_bd.sum(axis=0) / (NPTS * NPTS)
    return energy.astype(np.float32)


# revision 3
# speedup vs baseline: 1.2089x; 1.2089x over previous
"""Energy-distance kernel for 8 TRN2 NeuronCores (Bass/Tile).

Problem: x1 [16, 256, 128], x2 [16, 256, 128] fp32 ->
    energy[b] = mean_{i,j} ||x1[b,i]-x2[b,j]||_1
                - 0.5 * (mean ||x2-x2||_1 + mean ||x1-x1||_1)

Sharding: pure data parallel over the batch dim (2 batches per core).

Algorithm (per core): the O(N^2 D) pairwise L1 sums decompose per
feature dimension d into 1-D order statistics. For each (batch, d), the
merged array w = concat(u, v) (512 values, u = x1[b,:,d], v = x2[b,:,d])
is sorted with a 45-stage bitonic network on the vector engine (128 d's
in parallel across partitions). Origin (u vs v) rides in the key
mantissa LSB. With s = +-1 origin signs and G = inclusive cumsum(s):
    E(b,d) = sum(w) - 2 * sum(w * s * G)
    energy[b] = sum_d E(b,d) / (N*M)
The device returns R1 = sum(w*s*G) and R2 = sum(w) per (d, b); the host
combines them in float64.

FP16_KEYS=False sorts fp32 keys (rel err ~5e-5, matching the fp32
reference's own rounding); True sorts fp16 keys with the NB batch
arrays interleaved for DVE 2x mode (~30% faster, rel err ~6e-4).
"""
import numpy as np
from contextlib import ExitStack

from concourse import bacc, mybir, masks
from concourse.tile import TileContext
from concourse.bass_utils import run_bass_kernel_spmd

F32 = mybir.dt.float32
F16 = mybir.dt.float16
I32 = mybir.dt.int32
I16 = mybir.dt.int16
Alu = mybir.AluOpType

NCORES = 8
B = 16        # total batches
NB = 2        # batches per core
NPTS = 256    # points per set (N == M)
L = 2 * NPTS  # merged array length per (b, d)
WFREE = NB * L
D = 128

FP16_KEYS = False


def _bitonic_stages(length):
    k = 2
    while k <= length:
        yield ("rev", k)
        j = k // 4
        while j >= 1:
            yield ("reg", j)
            j //= 2
        k *= 2


def _build_nc(fp16=FP16_KEYS):
    interleave = fp16
    KD = F16 if fp16 else F32
    KI = I16 if fp16 else I32
    nc = bacc.Bacc("TRN2", target_bir_lowering=False, debug=False,
                   num_devices=NCORES)
    x1 = nc.dram_tensor("x1", [NB, NPTS, D], F32, kind="ExternalInput").ap()
    x2 = nc.dram_tensor("x2", [NB, NPTS, D], F32, kind="ExternalInput").ap()
    r1 = nc.dram_tensor("r1", [D, NB], F32, kind="ExternalOutput").ap()
    r2 = nc.dram_tensor("r2", [D, NB], F32, kind="ExternalOutput").ap()

    with TileContext(nc) as tc, ExitStack() as ctx:
        cpool = ctx.enter_context(tc.tile_pool(name="consts", bufs=1))
        wpool = ctx.enter_context(tc.tile_pool(name="work", bufs=1))
        xpool = ctx.enter_context(tc.tile_pool(name="xload", bufs=8))
        ppool = ctx.enter_context(tc.tile_pool(name="ps", bufs=2, space="PSUM"))

        ident = cpool.tile([128, 128], F32, tag="ident")
        masks.make_identity(nc, ident[:])

        wa = wpool.tile([128, WFREE], KD, tag="wa")
        wb = wpool.tile([128, WFREE], KD, tag="wb")

        def arr_slice(buf, b):
            if interleave:
                return buf[:].rearrange("p (l b) -> p l b", b=NB)[:, :, b]
            return buf[:, b * L:(b + 1) * L]

        # ---- load, transpose to [d, n], pack into wa ----
        for b in range(NB):
            ps = ppool.tile([128, L], F32, tag="psb")
            for half, x in ((0, x1), (1, x2)):
                for t in range(NPTS // 128):
                    xt = xpool.tile([128, 128], F32, tag="xt")
                    nc.sync.dma_start(out=xt[:],
                                      in_=x[b, t * 128:(t + 1) * 128, :])
                    c0 = (half * (NPTS // 128) + t) * 128
                    nc.tensor.transpose(ps[:, c0:c0 + 128], xt[:], ident[:])
            nc.scalar.copy(out=arr_slice(wa, b), in_=ps[:])

        # ---- LSB origin tag: u -> LSB 0, v -> LSB 1 ----
        wi = wa[:].bitcast(KI)
        if interleave:
            u_ap, v_ap = wi[:, :NB * NPTS], wi[:, NB * NPTS:]
        else:
            r = wi.rearrange("p (b h n) -> p b h n", b=NB, h=2)
            u_ap, v_ap = r[:, :, 0, :], r[:, :, 1, :]
        nc.vector.tensor_scalar(out=u_ap, in0=u_ap, scalar1=-2,
                                scalar2=None, op0=Alu.bitwise_and)
        nc.vector.tensor_scalar(out=v_ap, in0=v_ap, scalar1=1,
                                scalar2=None, op0=Alu.bitwise_or)

        # ---- bitonic sort (ping-pong wa <-> wb) ----
        IL = NB if interleave else 1
        bufs = [wa, wb]
        cur = 0
        for kind, p in _bitonic_stages(L):
            src = bufs[cur][:]
            dst = bufs[1 - cur][:]
            if kind == "reg":
                m = IL * p
                rs = src.rearrange("p (n t m) -> p n t m", t=2, m=m)
                rd = dst.rearrange("p (n t m) -> p n t m", t=2, m=m)
                lo_s, hi_s = rs[:, :, 0, :], rs[:, :, 1, :]
                lo_d, hi_d = rd[:, :, 0, :], rd[:, :, 1, :]
            elif interleave:
                rs = src.rearrange("p (n t x b) -> p n t x b",
                                   t=2, x=p // 2, b=IL)
                rd = dst.rearrange("p (n t x b) -> p n t x b",
                                   t=2, x=p // 2, b=IL)
                lo_s, hi_s = rs[:, :, 0, :, :], rs[:, :, 1, ::-1, :]
                lo_d, hi_d = rd[:, :, 0, :, :], rd[:, :, 1, ::-1, :]
            else:
                rs = src.rearrange("p (n t h) -> p n t h", t=2, h=p // 2)
                rd = dst.rearrange("p (n t h) -> p n t h", t=2, h=p // 2)
                lo_s, hi_s = rs[:, :, 0, :], rs[:, :, 1, ::-1]
                lo_d, hi_d = rd[:, :, 0, :], rd[:, :, 1, ::-1]
            nc.vector.tensor_tensor(out=lo_d, in0=lo_s, in1=hi_s, op=Alu.min)
            nc.vector.tensor_tensor(out=hi_d, in0=lo_s, in1=hi_s, op=Alu.max)
            cur = 1 - cur

        ws = bufs[cur]       # sorted keys (LSB-tagged)
        aux = bufs[1 - cur]  # reuse the other buffer for the mask

        # ---- origin signs s = 1 - 2*(w & 1) ----
        mi = aux[:].bitcast(KI)
        nc.vector.tensor_scalar(out=mi, in0=ws[:].bitcast(KI), scalar1=1,
                                scalar2=None, op0=Alu.bitwise_and)
        mf = wpool.tile([128, WFREE], KD, tag="mf")
        nc.vector.tensor_copy(out=mf[:], in_=mi)
        s_t = wpool.tile([128, WFREE], KD, tag="s_t")
        nc.vector.tensor_scalar(out=s_t[:], in0=mf[:], scalar1=-2.0,
                                scalar2=1.0, op0=Alu.mult, op1=Alu.add)

        # ---- G = inclusive cumsum(s) per (b) array ----
        g_t = wpool.tile([128, WFREE], KD, tag="g_t")
        for b in range(NB):
            nc.vector.tensor_tensor_scan(
                out=arr_slice(g_t, b), data0=arr_slice(s_t, b),
                data1=arr_slice(s_t, b),
                initial=0.0, op0=Alu.add, op1=Alu.bypass)

        # ---- R1 = sum(w*s*G), R2 = sum(w) per (d, b); reduce on ACT ----
        t1 = wpool.tile([128, WFREE], KD, tag="t1")
        nc.vector.tensor_mul(out=t1[:], in0=ws[:], in1=s_t[:])
        t2 = wpool.tile([128, WFREE], KD, tag="t2")
        nc.vector.tensor_mul(out=t2[:], in0=t1[:], in1=g_t[:])
        r1sb = wpool.tile([128, NB], F32, tag="r1sb")
        r2sb = wpool.tile([128, NB], F32, tag="r2sb")
        trash = wpool.tile([128, L], KD, tag="trash")
        Act = mybir.ActivationFunctionType
        for b in range(NB):
            nc.scalar.activation(out=trash[:], in_=arr_slice(ws, b),
                                 func=Act.Copy, accum_out=r2sb[:, b:b + 1])
        for b in range(NB):
            nc.scalar.activation(out=trash[:], in_=arr_slice(t2, b),
                                 func=Act.Copy, accum_out=r1sb[:, b:b + 1])

        nc.sync.dma_start(out=r1, in_=r1sb[:])
        nc.sync.dma_start(out=r2, in_=r2sb[:])

    nc.finalize()
    return nc


_NC_CACHE = {}


def _get_nc():
    if "nc" not in _NC_CACHE:
        _NC_CACHE["nc"] = _build_nc()
    return _NC_CACHE["nc"]


def kernel(x1, x2):
    """x1, x2: [16, 256, 128] fp32 -> energy distances [16] fp32."""
    x1 = np.ascontiguousarray(np.asarray(x1, dtype=np.float32))
    x2 = np.ascontiguousarray(np.asarray(x2, dtype=np.float32))
    nc = _get_nc()
    core_ids = list(range(NCORES))
    in_maps = []
    for c in core_ids:
        sl = slice(c * NB, (c + 1) * NB)
        in_maps.append({"x1": np.ascontiguousarray(x1[sl]),
                        "x2": np.ascontiguousarray(x2[sl])})
    try:
        res = run_bass_kernel_spmd(nc, in_maps, core_ids)
    except Exception:
        # transient device faults surface as runtime errors; retry once
        res = run_bass_kernel_spmd(nc, in_maps, core_ids)
    energy = np.zeros(B, dtype=np.float64)
    for c in core_ids:
        r1 = res.results[c]["r1"].astype(np.float64)  # [D, NB]
        r2 = res.results[c]["r2"].astype(np.float64)
        e_bd = r2 - 2.0 * r1
        energy[c * NB:(c + 1) * NB] = e_bd.sum(axis=0) / (NPTS * NPTS)
    return energy.astype(np.float32)
